# revision 65
# baseline (speedup 1.0000x reference)
"""GQA causal attention (RoPE) on 8 Trainium2 NeuronCores.

Sharding (tensor-parallel over heads, per the hint):
  core c owns q-heads {2c, 2c+1} and kv-head c//2.
  Each core computes its 2 heads' attention over the full sequence and a
  partial output projection out_c.T = wo[:, 128c:128c+128] @ att_c  (shape
  [1024, 4096], bf16); the final all-reduce over cores is the host-side
  unshard (f32 accumulate).

Device-side per core (v5 — cross-chunk software pipeline):
  The per-chunk group loop (one group = one key-block j, both heads) is the
  scheduling spine.  Into chunk n's groups we interleave, at fixed group
  indices: chunk n+1's x loads + projections + RoPE + V-transpose, and chunk
  n-1's softmax normalization + wo matmuls + output stores.  ScalarE does
  nothing but the 144 exp instructions, so it (the bottleneck engine at
  ~1 elem/cycle/lane) never idles at chunk boundaries; the PE never drains,
  keeping its p-state ramp warm.

  - All matmuls bf16 (1 cycle/row), fp32 PSUM.
  - RoPE in fp32 from PSUM: even/odd permutation folded into wq/wk host-side,
    sign-folded sin tile, partition-block swap via SBUF->SBUF DMA; bf16 out.
  - scores^T blocks [sk=128, sq=512]: K=64 matmuls; the two heads hit
    disjoint PE row groups -> concurrent (2nd matmul of each pair ~free).
  - Causal masks accumulated on the PE (identity matmul, -1e6 bf16 additive
    masks); grouped PSUM exp on ScalarE with below-diagonal columns trimmed
    via strided 3D access patterns.
  - AV with ones-augmented V^T so softmax denominators fall out of the same
    matmul (M=65); V^T via PE transpose-mode.
  - Normalization: denominators DMA-packed [1,512]->[128,4] so the DVE
    reciprocal runs on 8 free elems instead of 512; reciprocals broadcast
    across partitions by a K=2 select matmul; per-column scale on DVE.
  - Output partials stored bf16 (host all-reduce in f32).
"""
import numpy as np
import ml_dtypes
from contextlib import ExitStack

import concourse.bacc as bacc
import concourse.tile as tile
import concourse.mybir as mybir
from concourse.bass_utils import run_bass_kernel_spmd

DIM = 1024
N_HEADS = 16
N_KV = 4
HD = 64
SEQ = 4096
NCORES = 8

SQ = 512            # query-chunk (free dim of score blocks)
SK = 128            # key-chunk (partition dim of score blocks)
NQ = SEQ // SQ      # 8
NR = DIM // 128     # 8 contraction chunks for projections
NJ = SEQ // SK      # 32 key chunks
MASKVAL = -1.0e6

f32 = mybir.dt.float32
f32r = mybir.dt.float32r
bf16 = mybir.dt.bfloat16
FT = mybir.ActivationFunctionType

_CACHE = {}


def _emit(nc):
    xT = nc.dram_tensor("xT", [DIM, SEQ], bf16, kind="ExternalInput").ap()
    wq_l = nc.dram_tensor("wq_l", [128, DIM], bf16, kind="ExternalInput").ap()
    wkv_l = nc.dram_tensor("wkv_l", [128, DIM], bf16, kind="ExternalInput").ap()
    wo_l = nc.dram_tensor("wo_l", [128, DIM], bf16, kind="ExternalInput").ap()
    cos4_d = nc.dram_tensor("cos4", [128, SEQ], f32, kind="ExternalInput").ap()
    sin4_d = nc.dram_tensor("sin4", [128, SEQ], f32, kind="ExternalInput").ap()
    mask_d = nc.dram_tensor("mask", [128, 4 * SQ], bf16, kind="ExternalInput").ap()
    swp_d = nc.dram_tensor("swp", [128, 128], bf16, kind="ExternalInput").ap()
    on_d = nc.dram_tensor("ones32v2", [128, NJ], bf16, kind="ExternalInput").ap()
    id_d = nc.dram_tensor("ident", [128, 128], bf16, kind="ExternalInput").ap()
    sel_d = nc.dram_tensor("sel2", [128, 128], f32r, kind="ExternalInput").ap()
    out_d = nc.dram_tensor("out", [DIM, SEQ], bf16, kind="ExternalOutput").ap()

    with tile.TileContext(nc) as tc, ExitStack() as ctx:
        const = ctx.enter_context(tc.tile_pool(name="const", bufs=1))
        main = ctx.enter_context(tc.tile_pool(name="main", bufs=1))

        wq_sb = const.tile([128, DIM], bf16)
        wkv_sb = const.tile([128, DIM], bf16)
        wo_sb = const.tile([128, DIM], bf16)
        cos_sb = const.tile([128, SEQ], f32)
        sin_sb = const.tile([128, SEQ], f32)
        msk_sb = const.tile([128, 4 * SQ], bf16)
        id_sb = const.tile([128, 128], bf16)
        sel_sb = const.tile([128, 128], f32r)
        swp_sb = const.tile([128, 128], bf16)
        wsc = const.tile([128, 256], bf16)

        qrot = main.tile([128, SEQ], bf16)      # 2 heads d-major (rope'd)
        krot = main.tile([128, SEQ], bf16)      # k duplicated in both halves
        v_sb = main.tile([HD, SEQ], bf16)       # v d-major
        vt = main.tile([128, NJ, 128], bf16)    # v^T + ones column
        attS = main.tile([128, SEQ], bf16)      # stacked normalized att
        att1 = main.tile([HD, SEQ], bf16)       # head-1 att staging
        au0 = main.tile([HD + 1, SEQ], f32r)    # raw AV + denom staging, head 0
        au1 = main.tile([HD + 1, SEQ], f32r)    # head 1
        d2 = main.tile([66, SEQ], f32r)         # denom reciprocals (rows 64, 65)
        dsm = main.tile([128, NQ * 8], f32)     # packed denoms, 8 cols/chunk
        dr = main.tile([128, NQ * 8], f32)      # packed reciprocals
        rbs = main.tile([128, SQ], f32)         # recip1 staging (rows 64:128)
        rb1 = main.tile([HD, SQ], f32)          # recip1 at partitions 0:64

        with (
            tc.tile_pool(name="xp", bufs=2) as xp,
            tc.tile_pool(name="pp", bufs=1, space="PSUM") as pp,
            tc.tile_pool(name="rp", bufs=2) as rp,
            tc.tile_pool(name="sp", bufs=2, space="PSUM") as sp,
            tc.tile_pool(name="ap", bufs=1, space="PSUM") as ap,
            tc.tile_pool(name="ep", bufs=6) as ep,
            tc.tile_pool(name="op", bufs=4) as op,
        ):
            # ---------------- pipelined task pieces ----------------
            def t_xt(n, xtile, half, trig=True):
                """Batched x load for chunk n (one DMA per 4 r-blocks)."""
                s0 = n * SQ
                nc.sync.dma_start(
                    xtile[:, 4 * half:4 * half + 4, :],
                    xT[512 * half:512 * half + 512, s0:s0 + SQ].rearrange(
                        "(r p) f -> p r f", r=4))
                if half == 0 and trig:
                    nc.sync.dma_start(cos_sb[:, s0:s0 + SQ], cos4_d[:, s0:s0 + SQ])
                    nc.sync.dma_start(sin_sb[:, s0:s0 + SQ], sin4_d[:, s0:s0 + SQ])

            def t_proj(n, rpair, xtile, pq, pkv):
                """2 contraction steps of the q and kv projections for chunk n."""
                for r in (2 * rpair, 2 * rpair + 1):
                    xt = xtile[:, r, :]
                    nc.tensor.matmul(pq[:], wq_sb[:, 128 * r:128 * (r + 1)], xt,
                                     start=(r == 0), stop=(r == NR - 1))
                    nc.tensor.matmul(pkv[:], wkv_sb[:, 128 * r:128 * (r + 1)], xt,
                                     start=(r == 0), stop=(r == NR - 1))

            def t_ropeq(n, pq):
                # rope = pq*cos + swap32(pq*sin); the partition-block swap runs
                # on the PE as a permutation matmul (swp[m^32, m] = 1)
                s0 = n * SQ
                a_t = rp.tile([128, SQ], f32, tag="ta")
                c_t = rp.tile([128, SQ], bf16, tag="tc")
                nc.vector.tensor_mul(a_t[:], pq[:], cos_sb[:, s0:s0 + SQ])
                nc.vector.tensor_mul(c_t[:], pq[:], sin_sb[:, s0:s0 + SQ])
                bps = sp.tile([128, SQ], f32, tag="sc", name=f"bps_{n}")
                nc.tensor.matmul(bps[:], swp_sb[:], c_t[:], start=True, stop=True)
                nc.vector.tensor_add(qrot[:, s0:s0 + SQ], a_t[:], bps[:])

            def t_ropek(n, pkv):
                s0 = n * SQ
                ak = rp.tile([128, SQ], f32, tag="ta")
                ck = rp.tile([128, SQ], bf16, tag="tc")
                nc.vector.tensor_mul(ak[64:128, :], pkv[64:128, :],
                                     cos_sb[64:128, s0:s0 + SQ])
                # full 128 rows: rows 0:64 (= v*sin, unused) keep the swap
                # matmul's contraction free of uninitialized NaNs
                nc.vector.tensor_mul(ck[:], pkv[:], sin_sb[:, s0:s0 + SQ])
                bps = sp.tile([128, SQ], f32, tag="sc", name=f"bpk_{n}")
                nc.tensor.matmul(bps[:], swp_sb[:], ck[:], start=True, stop=True)
                nc.vector.tensor_add(krot[64:128, s0:s0 + SQ], ak[64:128, :],
                                     bps[64:128, :])
                nc.gpsimd.dma_start(krot[0:64, s0:s0 + SQ], krot[64:128, s0:s0 + SQ])

            def t_vt(n, pkv):
                s0 = n * SQ
                nc.vector.tensor_copy(v_sb[:, s0:s0 + SQ], pkv[0:64, :])
                for j in range(4 * n, 4 * n + 4):
                    pt = sp.tile([SK, HD], bf16, tag="sc", name=f"pt_{j}")
                    nc.tensor.transpose(pt[:], v_sb[:, SK * j:SK * (j + 1)],
                                        id_sb[0:HD, 0:HD])
                    nc.vector.tensor_copy(vt[:, j, 0:HD], pt[:])

            HQ = SQ // 2

            def t_stage(n):
                """End of chunk n: stage raw AV to SBUF, pack denominators.

                Packs [1, 256] -> [128, 2] per column-half so the reciprocal
                is cheap and the last chunk's endgame can run per-half."""
                s0 = n * SQ
                for h in (0, 1):
                    c0 = s0 + HQ * h
                    nc.vector.tensor_copy(au0[:, c0:c0 + HQ],
                                          av_tiles[n][0][:, HQ * h:HQ * h + HQ])
                    nc.vector.tensor_copy(au1[:, c0:c0 + HQ],
                                          av_tiles[n][1][:, HQ * h:HQ * h + HQ])
                    d0 = 8 * n + 4 * h
                    eng = nc.sync if n == NQ - 1 else nc.gpsimd
                    nc.gpsimd.dma_start(dsm[:, d0:d0 + 2],
                                        au0[HD:HD + 1, c0:c0 + HQ].bitcast(f32))
                    eng.dma_start(dsm[:, d0 + 2:d0 + 4],
                                  au1[HD:HD + 1, c0:c0 + HQ].bitcast(f32))

            def t_recip(k, h=None):
                """Reciprocal of chunk k's packed denominators + unpack."""
                s0 = k * SQ
                halves = (0, 1) if h is None else (h,)
                d0 = 8 * k + (0 if h is None else 4 * h)
                dw = 8 if h is None else 4
                nc.vector.reciprocal(dr[:, d0:d0 + dw], dsm[:, d0:d0 + dw])
                eng = nc.sync if k == NQ - 1 else nc.gpsimd
                for hh in halves:
                    c0 = s0 + HQ * hh
                    dh = 8 * k + 4 * hh
                    nc.gpsimd.dma_start(d2[64:65, c0:c0 + HQ].bitcast(f32),
                                        dr[:, dh:dh + 2])
                    eng.dma_start(d2[65:66, c0:c0 + HQ].bitcast(f32),
                                  dr[:, dh + 2:dh + 4])

            def t_bcmm(k, c0=0, cw=SQ):
                """Broadcast chunk k's reciprocals, normalize att."""
                s0 = k * SQ + c0
                bc = pp.tile([128, cw], f32, tag="pq", name=f"bc_{k}_{c0}")
                nc.tensor.matmul(bc[:], sel_sb[64:66, :], d2[64:66, s0:s0 + cw],
                                 start=True, stop=True)
                # rb1 DMA first: its flight overlaps the attS multiply
                nc.vector.tensor_copy(rbs[64:128, c0:c0 + cw], bc[64:128, :])
                eng = nc.sync if k == NQ - 1 else nc.gpsimd
                eng.dma_start(rb1[:, c0:c0 + cw], rbs[64:128, c0:c0 + cw])
                nc.vector.tensor_mul(attS[0:HD, s0:s0 + cw],
                                     au0[0:HD, s0:s0 + cw].bitcast(f32), bc[0:HD, :])
                nc.vector.tensor_mul(att1[:, s0:s0 + cw],
                                     au1[0:HD, s0:s0 + cw].bitcast(f32),
                                     rb1[:, c0:c0 + cw])
                nc.gpsimd.dma_start(attS[64:128, s0:s0 + cw], att1[:, s0:s0 + cw])

            def t_wo(k, mp, c0=0, cw=SQ):
                """Paired wo output-block matmuls + one batched bf16 store."""
                s0 = k * SQ + c0
                ot2 = op.tile([128, 2, cw], bf16, tag="ot")
                for i in (0, 1):
                    m = 2 * mp + i
                    pw = pp.tile([128, cw], f32, tag=("pkv" if i == 0 else "pq"),
                                 name=f"pw_{k}_{m}_{c0}")
                    nc.tensor.matmul(pw[:], wo_sb[:, 128 * m:128 * (m + 1)],
                                     attS[:, s0:s0 + cw], start=True, stop=True)
                    if k <= 2 or (k == NQ - 1 and i == 1):
                        # ScalarE has slack in the early chunks (and at the
                        # tail); DVE is the pacer there. Copy shares the Exp
                        # ACT table, so no table-reload cost.
                        nc.scalar.activation(ot2[:, i, :], pw[:], FT.Copy)
                    else:
                        nc.vector.tensor_copy(ot2[:, i, :], pw[:])
                eng = nc.sync if mp % 2 == 0 else nc.scalar
                eng.dma_start(
                    out_d[256 * mp:256 * (mp + 1), s0:s0 + cw].rearrange(
                        "(i p) f -> p i f", i=2),
                    ot2[:])

            # ---------------- PE clock warm-up under the initial loads ----------
            nc.gpsimd.memset(wsc[:], 0)
            for w in range(10):
                wps = sp.tile([128, 256], f32, tag="sc", name="warm")
                nc.tensor.matmul(wps[:], wsc[:, 0:128], wsc[:],
                                 start=True, stop=True)

            # ---------------- chunk 0 prelude (not pipelined) ----------------
            nc.sync.dma_start(wq_sb[:], wq_l[:])
            nc.gpsimd.dma_start(id_sb[:], id_d[:])
            nc.gpsimd.dma_start(swp_sb[:], swp_d[:])
            nc.gpsimd.dma_start(msk_sb[:], mask_d[:])
            xts0 = xp.tile([128, NR, SQ], bf16, tag="xt", name="xt_0")
            t_xt(0, xts0, 0, trig=False)
            nc.sync.dma_start(wkv_sb[:], wkv_l[:])
            t_xt(0, xts0, 1)
            s00 = 0
            nc.sync.dma_start(cos_sb[:, s00:s00 + SQ], cos4_d[:, s00:s00 + SQ])
            nc.sync.dma_start(sin_sb[:, s00:s00 + SQ], sin4_d[:, s00:s00 + SQ])
            nc.gpsimd.dma_start(vt[:, :, HD:HD + 1], on_d[:])
            pq0 = pp.tile([128, SQ], f32, tag="pq")
            pkv0 = pp.tile([128, SQ], f32, tag="pkv")
            for rp_i in range(4):
                t_proj(0, rp_i, xts0, pq0, pkv0)
            t_ropeq(0, pq0)
            t_ropek(0, pkv0)
            t_vt(0, pkv0)

            av_tiles = {}
            proj_state = {}
            # prefetch chunk 1's x + trig during the (otherwise dead) prelude
            st1 = proj_state[1] = {"loaded": True}
            st1["xts"] = xp.tile([128, NR, SQ], bf16, tag="xt", name="xt_1")
            t_xt(1, st1["xts"], 0, trig=True)
            t_xt(1, st1["xts"], 1)
            pend = []

            def flush_av():
                n_, j_, et_, dd_ = pend.pop(0)
                nsk_ = 4 * (n_ + 1)
                if n_ not in av_tiles:
                    av_tiles[n_] = [
                        ap.tile([HD + 1, SQ], f32, tag=f"av{h}", name=f"av{h}_{n_}")
                        for h in (0, 1)]
                av_ = av_tiles[n_]
                for h_ in (0, 1):
                    nc.tensor.matmul(
                        av_[h_][:, dd_:SQ], vt[:, j_, 0:HD + 1],
                        et_[:, h_ * SQ + dd_:(h_ + 1) * SQ],
                        start=(j_ == 0), stop=(j_ == nsk_ - 1),
                    )
                if j_ == nsk_ - 1:
                    # chunk n_ fully accumulated: stage AV + pack denominators
                    t_stage(n_)

            def make_tasks(n):
                # ---- interleave schedule: (group position, task) ----
                # Positions are compressed proportionally for short chunks so
                # prep work does not drain serially at early boundaries.
                G = 4 * (n + 1)
                def sp_(p):
                    return p if G >= 16 else (p * G) // 16
                tasks = []
                recip_pos = max(3, sp_(3))
                if n > 0:
                    tasks.append((recip_pos, lambda k=n - 1: t_recip(k)))
                if n == 0:
                    tasks.append((0, lambda: nc.sync.dma_start(wo_sb[:], wo_l[:])))
                    tasks.append((1, lambda: nc.sync.dma_start(sel_sb[:], sel_d[:])))
                proj_last = 0
                if n + 1 < NQ:
                    np1 = n + 1
                    st = proj_state.setdefault(np1, {})
                    def mk_load(half, np1=np1, st=st):
                        def f():
                            if not st.get("loaded"):
                                if half == 0:
                                    st["xts"] = xp.tile([128, NR, SQ], bf16,
                                                        tag="xt", name=f"xt_{np1}")
                                t_xt(np1, st["xts"], half, trig=(half == 0))
                            if half == 1:
                                st["pq"] = pp.tile([128, SQ], f32, tag="pq",
                                                   name=f"pq_{np1}")
                                st["pkv"] = pp.tile([128, SQ], f32, tag="pkv",
                                                    name=f"pkv_{np1}")
                        return f
                    tasks.append((sp_(1), mk_load(0)))
                    tasks.append((max(1, sp_(2)), mk_load(1)))
                    for rp_i in range(4):
                        proj_last = max(2, sp_(3 + rp_i))
                        tasks.append((proj_last,
                                      lambda r=rp_i, np1=np1, st=st:
                                      t_proj(np1, r, st["xts"], st["pq"], st["pkv"])))
                    # ropes/vt must be emitted at/after proj (PE order hazard)
                    rq_pos = max(sp_(7), proj_last)
                    rk_pos = max(sp_(8), rq_pos)
                    tasks.append((rq_pos, lambda np1=np1, st=st:
                                  t_ropeq(np1, st["pq"])))
                    tasks.append((rk_pos, lambda np1=np1, st=st:
                                  t_ropek(np1, st["pkv"])))
                else:
                    rk_pos = 0
                # bcmm/wo early: the wo phase is DVE-cast paced (~650ns/mm),
                # so it needs slack before the chunk boundary
                bcmm_pos = max(recip_pos + 2, sp_(6))
                if n > 0:
                    tasks.append((bcmm_pos, lambda k=n - 1: t_bcmm(k)))
                if n + 1 < NQ:
                    tasks.append((max(sp_(13), rk_pos), lambda np1=n + 1:
                                  t_vt(np1, proj_state[np1]["pkv"])))
                if n > 0:
                    for mp in range(4):
                        tasks.append((max(bcmm_pos + 2 + mp, sp_(9 + mp)),
                                      lambda k=n - 1, mp=mp: t_wo(k, mp)))
                return sorted(tasks, key=lambda t: t[0])

            for n in range(NQ):
                s0 = n * SQ
                nsk = 4 * (n + 1)
                tasks = make_tasks(n)
                ti = 0
                for gi in range(nsk):
                    j = gi
                    delta = SK * j - s0
                    dd = max(0, delta)
                    sc = sp.tile([128, 2 * SQ], f32, tag="sc")
                    for h in (0, 1):
                        nc.tensor.matmul(
                            sc[:, h * SQ + dd:(h + 1) * SQ],
                            krot[64 * h:64 * h + 64, SK * j:SK * (j + 1)],
                            qrot[64 * h:64 * h + 64, s0 + dd:s0 + SQ],
                            start=True, stop=(delta < 0),
                        )
                    if delta >= 0:
                        db = (delta // SK) * SQ + dd
                        for h in (0, 1):
                            nc.tensor.matmul(sc[:, h * SQ + delta:h * SQ + delta + SK],
                                             id_sb[:], msk_sb[:, db:db + SK],
                                             start=False, stop=True)
                    et = ep.tile([128, 2 * SQ], bf16, tag="et")
                    nc.scalar.activation(et[:, dd:2 * SQ], sc[:, dd:2 * SQ],
                                         FT.Exp, scale=0.125)
                    pend.append((n, j, et, dd))
                    if len(pend) > 3:
                        flush_av()
                    # interleaved cross-chunk tasks
                    while ti < len(tasks) and tasks[ti][0] <= gi:
                        tasks[ti][1]()
                        ti += 1
                while ti < len(tasks):
                    tasks[ti][1]()
                    ti += 1

            while pend:
                flush_av()

            # ---- final chunk endgame (pipelined per column-half) ----
            L = NQ - 1
            t_recip(L, h=0)
            t_recip(L, h=1)
            t_bcmm(L, 0, HQ)
            t_bcmm(L, HQ, HQ)
            for mp in range(4):
                t_wo(L, mp, 0, HQ)
            for mp in range(4):
                t_wo(L, mp, HQ, HQ)


def _build():
    if "nc" in _CACHE:
        return _CACHE["nc"]
    nc = bacc.Bacc("TRN2", target_bir_lowering=False, debug=False, num_devices=NCORES)
    _emit(nc)
    nc.compile()
    _CACHE["nc"] = nc
    return nc


def _host_inputs(x, freqs_cos, freqs_sin, wq, wk, wv, wo):
    x = np.asarray(x, np.float32)
    freqs_cos = np.asarray(freqs_cos, np.float32)
    freqs_sin = np.asarray(freqs_sin, np.float32)
    wq = np.asarray(wq, np.float32)
    wk = np.asarray(wk, np.float32)
    wv = np.asarray(wv, np.float32)
    wo = np.asarray(wo, np.float32)

    xT = np.ascontiguousarray(x[0].T).astype(ml_dtypes.bfloat16)   # [1024, 4096]
    cosT = freqs_cos.T                                             # [32, 4096]
    sinT = freqs_sin.T
    cos4 = np.ascontiguousarray(np.tile(cosT, (4, 1)))             # [128, 4096]
    sin4 = np.ascontiguousarray(
        np.concatenate([sinT, -sinT, sinT, -sinT], axis=0))

    # diagonal-block causal masks for delta in {0,128,256,384}
    p = np.arange(SK)[:, None]
    f = np.arange(SQ)[None, :]
    mask = np.concatenate(
        [np.where(SK * d + p <= f, 0.0, MASKVAL) for d in range(4)],
        axis=1).astype(ml_dtypes.bfloat16)                         # [128, 2048]

    ones32 = np.ones((128, NJ), dtype=ml_dtypes.bfloat16)
    ident = np.eye(128, dtype=ml_dtypes.bfloat16)
    swp = np.zeros((128, 128), dtype=ml_dtypes.bfloat16)
    for m in range(128):
        swp[m ^ 32, m] = 1.0
    sel2 = np.zeros((128, 128), dtype=np.float32)
    sel2[64, 0:64] = 1.0
    sel2[65, 64:128] = 1.0

    perm = np.concatenate([np.arange(0, HD, 2), np.arange(1, HD, 2)])

    def fold(w):  # [128(m), 1024(d)] -> lhsT layout [128(p), 8r*128+m]
        return np.ascontiguousarray(
            w.reshape(128, NR, 128).transpose(2, 1, 0).reshape(128, DIM)
        ).astype(ml_dtypes.bfloat16)

    in_maps = []
    for c in range(NCORES):
        g = c // 2
        wq_c = wq[128 * c:128 * (c + 1)].reshape(2, HD, DIM)[:, perm, :].reshape(128, DIM)
        wk_g = wk[HD * g:HD * (g + 1)][perm]
        wv_g = wv[HD * g:HD * (g + 1)]
        wkv_c = np.concatenate([wv_g, wk_g], axis=0)        # v rows 0:64, k rows 64:128
        wo_c = np.ascontiguousarray(wo[:, 128 * c:128 * (c + 1)].T).astype(
            ml_dtypes.bfloat16)                              # [128(j), 1024(o)]
        in_maps.append({
            "xT": xT,
            "wq_l": fold(wq_c),
            "wkv_l": fold(wkv_c),
            "wo_l": wo_c,
            "cos4": cos4,
            "sin4": sin4,
            "mask": mask,
            "swp": swp,
            "ones32v2": ones32,
            "ident": ident,
            "sel2": sel2,
        })
    return in_maps


def kernel(x, freqs_cos, freqs_sin, wq, wk, wv, wo, _trace=False, _trace_kwargs=None):
    nc = _build()
    in_maps = _host_inputs(x, freqs_cos, freqs_sin, wq, wk, wv, wo)
    kw = {}
    if _trace:
        kw.update(trace=True, **(_trace_kwargs or {}))
    res = run_bass_kernel_spmd(nc, in_maps, core_ids=list(range(NCORES)), **kw)
    acc = np.zeros((DIM, SEQ), np.float32)
    for c in range(NCORES):
        acc += res.results[c]["out"].astype(np.float32)
    out = np.ascontiguousarray(acc.T).reshape(1, SEQ, DIM)
    if _trace:
        kernel._last_results = res
    return out
